# revision 1
# baseline (speedup 1.0000x reference)
"""Trainium2 Bass kernel for a dense (length-1 sequence) Mamba block.

The reference computation reduces algebraically to:
    z   = x @ in_w                                  # (B, d_inner)
    g   = silu(z * c + b_eff)                       # per-channel scale/bias
    out = g @ out_w + out_b                         # (B, d_model)
with
    c     = conv_w[:, -1] + softplus(dt) * sum(B*C, -1) + Dp
    b_eff = (in_b * c) + conv_b
(c, b_eff are tiny per-channel vectors, computed on host.)

Strategy: data-parallel over 8 NeuronCores (batch 32768 -> 8 x 4096).
Per core, batch is processed in tiles of BT rows:
  phase T : PE-transpose x tiles into xT [d_model, BT] layout
  phase M1: z^T[di, b] accumulated over d_model via float32r (FP22) matmuls
            with in_w tiles as the stationary operand; Silu fused on ScalarE
            with per-partition scale/bias -> g [di, b]
  phase M2: out[b, dm] accumulated over d_inner with g slices as the
            stationary operand and out_w tiles moving (natural output
            layout; no output transpose needed); out_b added on drain.
"""

import numpy as np

import concourse.bass as bass
import concourse.tile as tile
from concourse import bacc, mybir
from concourse.bass_utils import run_bass_kernel_spmd

P = 128
B_FULL = 32768
DM = 2048
DI = 4096
N_CORES = 8
BS = B_FULL // N_CORES  # rows per core

F32 = mybir.dt.float32
F32R = mybir.dt.float32r
BF16 = mybir.dt.bfloat16
SILU = mybir.ActivationFunctionType.Silu


# float32r (FP22) tensors: same fp32 bytes in DRAM/numpy, but instructions
# producing them round to FP22 so the full-speed reduced-precision matmul
# path can consume them (walrus verifier requirement).


def build_nc(cfg):
    """Build the per-core Bass module. cfg: dict(BT=..., g_bf16=..., ow_bf16=...)"""
    BT = cfg["BT"]
    g_dt = BF16 if cfg["g_bf16"] else F32R
    ow_dt = BF16 if cfg["ow_bf16"] else F32R

    NBT = BS // BT          # batch tiles per core
    NB_SUB = BT // P        # 128-row subtiles per batch tile
    KT = DM // P            # k-tiles for matmul 1
    NDI = DI // P           # d_inner chunks of 128
    NDM = DM // 512         # d_model chunks of 512
    H = BT // 512           # moving-dim halves for matmul 1
    GRP = 4                 # psum banks used by M2 accumulation
    NGRP = NB_SUB // GRP

    nc = bacc.Bacc("TRN2", target_bir_lowering=False, debug=False,
                   num_devices=N_CORES)

    x_d = nc.dram_tensor("x", [BS, DM], F32R, kind="ExternalInput").ap()
    iw_d = nc.dram_tensor("iw", [DM, DI], F32R, kind="ExternalInput").ap()
    ow_d = nc.dram_tensor("ow", [DI, DM], ow_dt, kind="ExternalInput").ap()
    c_d = nc.dram_tensor("cpb", [P, NDI], F32, kind="ExternalInput").ap()
    b_d = nc.dram_tensor("bpb", [P, NDI], F32, kind="ExternalInput").ap()
    ob_d = nc.dram_tensor("ob", [P, DM], F32, kind="ExternalInput").ap()
    id_d = nc.dram_tensor("ident", [P, P], F32R, kind="ExternalInput").ap()
    out_d = nc.dram_tensor("out", [BS, DM], F32, kind="ExternalOutput").ap()

    DIG = 4                 # d_inner chunks per out_w DMA batch
    with tile.TileContext(nc) as tc:
        with (
            tc.tile_pool(name="const", bufs=1) as const,
            tc.tile_pool(name="xnat", bufs=2) as xnat,
            tc.tile_pool(name="xT", bufs=1) as xTp,
            tc.tile_pool(name="g", bufs=1) as gp,
            tc.tile_pool(name="iw", bufs=3) as iwp,
            tc.tile_pool(name="ow", bufs=3) as owp,
            tc.tile_pool(name="osb", bufs=2) as osbp,
            tc.tile_pool(name="psZ", bufs=3, space="PSUM") as psZ,
            tc.tile_pool(name="psO", bufs=5, space="PSUM") as psO,
        ):
            ident = const.tile([P, P], F32R)
            nc.sync.dma_start(ident[:], id_d)
            c_sb = const.tile([P, NDI], F32)
            nc.sync.dma_start(c_sb[:], c_d)
            b_sb = const.tile([P, NDI], F32)
            nc.sync.dma_start(b_sb[:], b_d)
            ob_sb = const.tile([P, DM], F32)
            nc.sync.dma_start(ob_sb[:], ob_d)

            xT = xTp.tile([P, KT, BT], F32R)
            g = gp.tile([P, NDI, BT], g_dt)

            def emit_T(t, bs):
                """Transpose one 128-row block of x[t] into xT."""
                xn = xnat.tile([P, DM], F32R, tag="xn", name="xn")
                nc.gpsimd.dma_start(xn[:], x_d[t * BT + bs * P:
                                               t * BT + (bs + 1) * P, :])
                for kq in range(KT // 4):
                    pst = psO.tile([P, 4, P], F32R, tag="ps_o", name="pst")
                    for q in range(4):
                        kt = kq * 4 + q
                        nc.tensor.transpose(
                            pst[:, q, :], xn[:, kt * P:(kt + 1) * P],
                            ident[:])
                    nc.vector.tensor_copy(
                        out=xT[:, kq * 4:(kq + 1) * 4, bs * P:(bs + 1) * P],
                        in_=pst[:])

            for t in range(NBT):
                if t == 0:
                    # prologue: transpose the first batch tile up front
                    for bs in range(NB_SUB):
                        emit_T(0, bs)

                # ---- phase M1: z^T = in_w^T @ x^T ; g = silu(z*c + b) ----
                for di in range(NDI):
                    iw_t = iwp.tile([P, KT, P], F32R)
                    nc.scalar.dma_start(
                        iw_t[:],
                        iw_d[:, di * P:(di + 1) * P].rearrange(
                            "(kt p) m -> p kt m", p=P))
                    zps = [psZ.tile([P, 512], F32, tag="zp", name=f"zp_{h}")
                           for h in range(H)]
                    for kt in range(KT):
                        for h in range(H):
                            nc.tensor.matmul(
                                zps[h][:],
                                iw_t[:, kt, :],
                                xT[:, kt, h * 512:(h + 1) * 512],
                                start=(kt == 0), stop=(kt == KT - 1))
                    for h in range(H):
                        nc.scalar.activation(
                            g[:, di, h * 512:(h + 1) * 512], zps[h][:], SILU,
                            bias=b_sb[:, di:di + 1], scale=c_sb[:, di:di + 1])

                # ---- phase M2: out = g^T @ out_w + out_b ----
                # T-units for batch tile t+1 interleaved after each psum
                # group: transposes fill PE slack, x DMAs spread over the
                # whole M2 window.
                ui = 0
                for dmc in range(NDM):
                    for grp in range(NGRP):
                        ops = [psO.tile([P, 512], F32, tag="ps_o",
                                        name=f"ops_{j}")
                               for j in range(GRP)]
                        for dg in range(NDI // DIG):
                            ow_t = owp.tile([P, DIG, 512], ow_dt)
                            nc.sync.dma_start(
                                ow_t[:],
                                ow_d[dg * DIG * P:(dg + 1) * DIG * P,
                                     dmc * 512:(dmc + 1) * 512].rearrange(
                                         "(s p) n -> p s n", p=P))
                            for s in range(DIG):
                                di = dg * DIG + s
                                for j in range(GRP):
                                    bs = grp * GRP + j
                                    nc.tensor.matmul(
                                        ops[j][:],
                                        g[:, di, bs * P:(bs + 1) * P],
                                        ow_t[:, s, :],
                                        start=(di == 0),
                                        stop=(di == NDI - 1))
                        osb = osbp.tile([P, GRP, 512], F32)
                        for j in range(GRP):
                            nc.vector.tensor_tensor(
                                osb[:, j, :], ops[j][:],
                                ob_sb[:, dmc * 512:(dmc + 1) * 512],
                                mybir.AluOpType.add)
                        r0 = t * BT + grp * GRP * P
                        nc.scalar.dma_start(
                            out_d[r0:r0 + GRP * P,
                                  dmc * 512:(dmc + 1) * 512].rearrange(
                                      "(s p) n -> p s n", p=P),
                            osb[:])
                        if t + 1 < NBT and ui < NB_SUB:
                            emit_T(t + 1, ui)
                            ui += 1
    nc.compile()
    return nc


_NC_CACHE = {}


def _get_nc(key):
    if key not in _NC_CACHE:
        cfg = dict(BT=key[0], g_bf16=key[1], ow_bf16=key[2])
        _NC_CACHE[key] = build_nc(cfg)
    return _NC_CACHE[key]


# default config: fp32r matmul-1, bf16 g + out_w for matmul-2, BT=1024
CONFIG = (1024, True, True)


def _softplus(v):
    return np.logaddexp(0.0, v)


def kernel(x, in_w, in_b, conv_w, conv_b, A_log, B, C, Dp, dt, out_w, out_b):
    x = np.asarray(x, dtype=np.float32)
    in_w = np.ascontiguousarray(np.asarray(in_w, dtype=np.float32))
    out_w = np.asarray(out_w, dtype=np.float32)

    # host precompute of the per-channel SSM/conv collapse
    c = (np.asarray(conv_w, np.float32)[:, -1]
         + _softplus(np.asarray(dt, np.float32))
         * np.sum(np.asarray(B, np.float32) * np.asarray(C, np.float32), -1)
         + np.asarray(Dp, np.float32))
    b_eff = np.asarray(in_b, np.float32) * c + np.asarray(conv_b, np.float32)

    # [128, DI//128] partition-major layouts for per-partition scale/bias
    c_pb = np.ascontiguousarray(c.reshape(DI // P, P).T)
    b_pb = np.ascontiguousarray(b_eff.reshape(DI // P, P).T)
    ob_rep = np.ascontiguousarray(
        np.broadcast_to(np.asarray(out_b, np.float32), (P, DM)))

    key = CONFIG
    nc = _get_nc(key)
    if key[2]:
        import ml_dtypes
        ow_arr = out_w.astype(ml_dtypes.bfloat16)
    else:
        ow_arr = np.ascontiguousarray(out_w)

    in_maps = []
    for i in range(N_CORES):
        in_maps.append({
            "x": np.ascontiguousarray(x[i * BS:(i + 1) * BS]),
            "iw": in_w,
            "ow": ow_arr,
            "cpb": c_pb,
            "bpb": b_pb,
            "ob": ob_rep,
            "ident": np.eye(P, dtype=np.float32),
        })
    out = np.empty((B_FULL, DM), dtype=np.float32)
    try:
        res = run_bass_kernel_spmd(nc, in_maps, core_ids=list(range(N_CORES)))
        for i in range(N_CORES):
            out[i * BS:(i + 1) * BS] = res.results[i]["out"]
    except Exception:
        # The accelerator occasionally hits a transient unrecoverable fault
        # that poisons this process's PJRT client; a fresh process recovers.
        # Retry the device execution in a subprocess.
        _run_in_subprocess(in_maps, out)
    return out


def _run_in_subprocess(in_maps, out):
    import pickle
    import subprocess
    import sys
    import tempfile

    with tempfile.TemporaryDirectory() as td:
        in_path = f"{td}/in.pkl"
        out_path = f"{td}/out.npy"
        with open(in_path, "wb") as f:
            pickle.dump({"config": CONFIG, "in_maps": in_maps}, f,
                        protocol=pickle.HIGHEST_PROTOCOL)
        for attempt in range(3):
            r = subprocess.run(
                [sys.executable, __file__, "--worker", in_path, out_path],
                capture_output=True)
            if r.returncode == 0:
                break
            if attempt == 2:
                raise RuntimeError(
                    f"device worker failed 3x: {r.stderr[-2000:]!r}")
        out[:] = np.load(out_path)


def _worker_main(in_path, out_path):
    import pickle
    with open(in_path, "rb") as f:
        job = pickle.load(f)
    nc = _get_nc(tuple(job["config"]))
    res = run_bass_kernel_spmd(nc, job["in_maps"],
                               core_ids=list(range(N_CORES)))
    out = np.empty((B_FULL, DM), dtype=np.float32)
    for i in range(N_CORES):
        out[i * BS:(i + 1) * BS] = res.results[i]["out"]
    np.save(out_path, out)


if __name__ == "__main__":
    import sys as _sys
    if len(_sys.argv) == 4 and _sys.argv[1] == "--worker":
        _worker_main(_sys.argv[2], _sys.argv[3])



# revision 3
# speedup vs baseline: 1.0891x; 1.0891x over previous
"""Trainium2 Bass kernel for a dense (length-1 sequence) Mamba block.

The reference computation reduces algebraically to:
    z   = x @ in_w                                  # (B, d_inner)
    g   = silu(z * c + b_eff)                       # per-channel scale/bias
    out = g @ out_w + out_b                         # (B, d_model)
with
    c     = conv_w[:, -1] + softplus(dt) * sum(B*C, -1) + Dp
    b_eff = (in_b * c) + conv_b
(c, b_eff are tiny per-channel vectors, computed on host.)

Strategy: data-parallel over 8 NeuronCores (batch 32768 -> 8 x 4096).
All matmul operands are bf16 (validated: end-to-end rel err ~4e-3 vs the
2e-2 budget), which enables fast-weight-load on every LDWEIGHTS and
halves HBM traffic. x is transposed on the HOST so the kernel streams
xT [d_model, batch] tiles straight from DRAM -- no PE transposes at all.

Per core, batch is processed in tiles of BT rows:
  M1: z^T[di, b] accumulated over d_model with in_w tiles stationary;
      Silu fused on ScalarE with per-partition scale/bias -> g [di, b]
  M2: out[b, dm] accumulated over d_inner with g slices stationary and
      out_w tiles moving (natural output layout); out_b added on the
      PSUM drain; result stored/DMA'd as bf16 and upcast on host.
"""

import numpy as np

import concourse.bass as bass
import concourse.tile as tile
from concourse import bacc, mybir
from concourse.bass_utils import run_bass_kernel_spmd

P = 128
B_FULL = 32768
DM = 2048
DI = 4096
N_CORES = 8
BS = B_FULL // N_CORES  # rows per core

F32 = mybir.dt.float32
BF16 = mybir.dt.bfloat16
SILU = mybir.ActivationFunctionType.Silu


def build_nc(cfg):
    """Build the per-core Bass module. cfg: dict(BT=...)"""
    BT = cfg["BT"]

    NBT = BS // BT          # batch tiles per core
    NB_SUB = BT // P        # 128-row subtiles per batch tile
    KT = DM // P            # k-tiles for matmul 1
    NDI = DI // P           # d_inner chunks of 128
    NDM = DM // 512         # d_model chunks of 512
    H = BT // 512           # moving-dim chunks for matmul 1
    GRP = 4                 # psum banks used by M2 accumulation
    NGRP = NB_SUB // GRP
    DIG = 4                 # d_inner chunks per out_w DMA batch

    nc = bacc.Bacc("TRN2", target_bir_lowering=False, debug=False,
                   num_devices=N_CORES)

    xt_d = nc.dram_tensor("xt", [DM, BS], BF16, kind="ExternalInput").ap()
    iw_d = nc.dram_tensor("iw", [DM, DI], BF16, kind="ExternalInput").ap()
    ow_d = nc.dram_tensor("ow", [DI, DM], BF16, kind="ExternalInput").ap()
    c_d = nc.dram_tensor("cpb", [P, NDI], F32, kind="ExternalInput").ap()
    b_d = nc.dram_tensor("bpb", [P, NDI], F32, kind="ExternalInput").ap()
    ob_d = nc.dram_tensor("ob", [P, DM], F32, kind="ExternalInput").ap()
    out_d = nc.dram_tensor("out", [BS, DM], BF16, kind="ExternalOutput").ap()

    with tile.TileContext(nc) as tc:
        with (
            tc.tile_pool(name="const", bufs=1) as const,
            tc.tile_pool(name="xk", bufs=2) as xkp,
            tc.tile_pool(name="g", bufs=1) as gp,
            tc.tile_pool(name="iw", bufs=3) as iwp,
            tc.tile_pool(name="ow", bufs=3) as owp,
            tc.tile_pool(name="osb", bufs=2) as osbp,
            tc.tile_pool(name="psZ", bufs=4, space="PSUM") as psZ,
            tc.tile_pool(name="psO", bufs=4, space="PSUM") as psO,
        ):
            c_sb = const.tile([P, NDI], F32)
            nc.sync.dma_start(c_sb[:], c_d)
            b_sb = const.tile([P, NDI], F32)
            nc.sync.dma_start(b_sb[:], b_d)
            ob_sb = const.tile([P, DM], F32)
            nc.sync.dma_start(ob_sb[:], ob_d)

            g = gp.tile([P, NDI, BT], BF16)

            for t in range(NBT):
                # ---- xT tiles for this batch tile (per-kt for fine deps) --
                xk = []
                for kt in range(KT):
                    xt_t = xkp.tile([P, BT], BF16, tag=f"xk{kt}",
                                    name=f"xk{kt}")
                    nc.gpsimd.dma_start(
                        xt_t[:], xt_d[kt * P:(kt + 1) * P,
                                      t * BT:(t + 1) * BT])
                    xk.append(xt_t)

                # ---- phase M1: z^T = in_w^T @ x^T ; g = silu(z*c + b) ----
                for di in range(NDI):
                    iw_t = iwp.tile([P, KT, P], BF16)
                    nc.scalar.dma_start(
                        iw_t[:],
                        iw_d[:, di * P:(di + 1) * P].rearrange(
                            "(kt p) m -> p kt m", p=P))
                    zps = [psZ.tile([P, 512], F32, tag="zp", name=f"zp_{h}")
                           for h in range(H)]
                    for kt in range(KT):
                        for h in range(H):
                            nc.tensor.matmul(
                                zps[h][:],
                                iw_t[:, kt, :],
                                xk[kt][:, h * 512:(h + 1) * 512],
                                start=(kt == 0), stop=(kt == KT - 1))
                    for h in range(H):
                        nc.scalar.activation(
                            g[:, di, h * 512:(h + 1) * 512], zps[h][:], SILU,
                            bias=b_sb[:, di:di + 1], scale=c_sb[:, di:di + 1])

                # ---- phase M2: out = g^T @ out_w + out_b ----
                for dmc in range(NDM):
                    for grp in range(NGRP):
                        ops = [psO.tile([P, 512], F32, tag="ps_o",
                                        name=f"ops_{j}")
                               for j in range(GRP)]
                        for dg in range(NDI // DIG):
                            ow_t = owp.tile([P, DIG, 512], BF16)
                            nc.sync.dma_start(
                                ow_t[:],
                                ow_d[dg * DIG * P:(dg + 1) * DIG * P,
                                     dmc * 512:(dmc + 1) * 512].rearrange(
                                         "(s p) n -> p s n", p=P))
                            for s in range(DIG):
                                di = dg * DIG + s
                                for j in range(GRP):
                                    bs = grp * GRP + j
                                    nc.tensor.matmul(
                                        ops[j][:],
                                        g[:, di, bs * P:(bs + 1) * P],
                                        ow_t[:, s, :],
                                        start=(di == 0),
                                        stop=(di == NDI - 1))
                        osb = osbp.tile([P, GRP, 512], BF16)
                        for j in range(GRP):
                            nc.vector.tensor_tensor(
                                osb[:, j, :], ops[j][:],
                                ob_sb[:, dmc * 512:(dmc + 1) * 512],
                                mybir.AluOpType.add)
                        r0 = t * BT + grp * GRP * P
                        nc.scalar.dma_start(
                            out_d[r0:r0 + GRP * P,
                                  dmc * 512:(dmc + 1) * 512].rearrange(
                                      "(s p) n -> p s n", p=P),
                            osb[:])
    nc.compile()
    return nc


_NC_CACHE = {}


def _get_nc(key):
    if key not in _NC_CACHE:
        cfg = dict(BT=key[0])
        _NC_CACHE[key] = build_nc(cfg)
    return _NC_CACHE[key]


CONFIG = (1024,)


def _softplus(v):
    return np.logaddexp(0.0, v)


def prepare_in_maps(inputs):
    """Host-side prep: per-core input dicts (shared by kernel() and tests)."""
    import ml_dtypes
    bf = ml_dtypes.bfloat16

    x = np.asarray(inputs["x"], dtype=np.float32)
    in_w = np.asarray(inputs["in_w"], dtype=np.float32)
    out_w = np.asarray(inputs["out_w"], dtype=np.float32)

    # host precompute of the per-channel SSM/conv collapse
    c = (np.asarray(inputs["conv_w"], np.float32)[:, -1]
         + _softplus(np.asarray(inputs["dt"], np.float32))
         * np.sum(np.asarray(inputs["B"], np.float32)
                  * np.asarray(inputs["C"], np.float32), -1)
         + np.asarray(inputs["Dp"], np.float32))
    b_eff = (np.asarray(inputs["in_b"], np.float32) * c
             + np.asarray(inputs["conv_b"], np.float32))

    # [128, DI//128] partition-major layouts for per-partition scale/bias
    c_pb = np.ascontiguousarray(c.reshape(DI // P, P).T)
    b_pb = np.ascontiguousarray(b_eff.reshape(DI // P, P).T)
    ob_rep = np.ascontiguousarray(
        np.broadcast_to(np.asarray(inputs["out_b"], np.float32), (P, DM)))

    iw_bf = in_w.astype(bf)
    ow_bf = out_w.astype(bf)
    x_bf = x.astype(bf)

    in_maps = []
    for i in range(N_CORES):
        in_maps.append({
            "xt": np.ascontiguousarray(x_bf[i * BS:(i + 1) * BS].T),
            "iw": iw_bf,
            "ow": ow_bf,
            "cpb": c_pb,
            "bpb": b_pb,
            "ob": ob_rep,
        })
    return in_maps


def kernel(x, in_w, in_b, conv_w, conv_b, A_log, B, C, Dp, dt, out_w, out_b):
    in_maps = prepare_in_maps(dict(
        x=x, in_w=in_w, in_b=in_b, conv_w=conv_w, conv_b=conv_b,
        A_log=A_log, B=B, C=C, Dp=Dp, dt=dt, out_w=out_w, out_b=out_b))
    nc = _get_nc(CONFIG)
    out = np.empty((B_FULL, DM), dtype=np.float32)
    try:
        res = run_bass_kernel_spmd(nc, in_maps, core_ids=list(range(N_CORES)))
        for i in range(N_CORES):
            out[i * BS:(i + 1) * BS] = res.results[i]["out"].astype(np.float32)
    except Exception:
        # The accelerator occasionally hits a transient unrecoverable fault
        # that poisons this process's PJRT client; a fresh process recovers.
        # Retry the device execution in a subprocess.
        _run_in_subprocess(in_maps, out)
    return out


def _run_in_subprocess(in_maps, out):
    import pickle
    import subprocess
    import sys
    import tempfile

    with tempfile.TemporaryDirectory() as td:
        in_path = f"{td}/in.pkl"
        out_path = f"{td}/out.npy"
        with open(in_path, "wb") as f:
            pickle.dump({"config": CONFIG, "in_maps": in_maps}, f,
                        protocol=pickle.HIGHEST_PROTOCOL)
        for attempt in range(3):
            r = subprocess.run(
                [sys.executable, __file__, "--worker", in_path, out_path],
                capture_output=True)
            if r.returncode == 0:
                break
            if attempt == 2:
                raise RuntimeError(
                    f"device worker failed 3x: {r.stderr[-2000:]!r}")
        out[:] = np.load(out_path)


def _worker_main(in_path, out_path):
    import pickle
    with open(in_path, "rb") as f:
        job = pickle.load(f)
    nc = _get_nc(tuple(job["config"]))
    res = run_bass_kernel_spmd(nc, job["in_maps"],
                               core_ids=list(range(N_CORES)))
    out = np.empty((B_FULL, DM), dtype=np.float32)
    for i in range(N_CORES):
        out[i * BS:(i + 1) * BS] = res.results[i]["out"].astype(np.float32)
    np.save(out_path, out)


if __name__ == "__main__":
    import sys as _sys
    if len(_sys.argv) == 4 and _sys.argv[1] == "--worker":
        _worker_main(_sys.argv[2], _sys.argv[3])


# revision 7
# speedup vs baseline: 1.1008x; 1.0107x over previous
"""Trainium2 Bass kernel for a dense (length-1 sequence) Mamba block.

The reference computation reduces algebraically to:
    z   = x @ in_w                                  # (B, d_inner)
    g   = silu(z * c + b_eff)                       # per-channel scale/bias
    out = g @ out_w + out_b                         # (B, d_model)
with
    c     = conv_w[:, -1] + softplus(dt) * sum(B*C, -1) + Dp
    b_eff = (in_b * c) + conv_b
(c, b_eff are tiny per-channel vectors, computed on host.)

Strategy: data-parallel over 8 NeuronCores (batch 32768 -> 8 x 4096).
All matmul operands are bf16 (validated: end-to-end rel err ~4e-3 vs the
2e-2 budget), which enables fast-weight-load on every LDWEIGHTS and
halves HBM traffic. x is transposed on the HOST so the kernel streams
xT [d_model, batch] tiles straight from DRAM -- no PE transposes at all.

Per core, batch is processed in tiles of BT rows:
  M1: z^T[di, b] accumulated over d_model with in_w tiles stationary;
      Silu fused on ScalarE with per-partition scale/bias -> g [di, b]
  M2: out[b, dm] accumulated over d_inner with g slices stationary and
      out_w tiles moving (natural output layout); out_b added on the
      PSUM drain; result stored/DMA'd as bf16 and upcast on host.
"""

import numpy as np

import concourse.bass as bass
import concourse.tile as tile
from concourse import bacc, mybir
from concourse.bass_utils import run_bass_kernel_spmd

P = 128
B_FULL = 32768
DM = 2048
DI = 4096
N_CORES = 8
BS = B_FULL // N_CORES  # rows per core

F32 = mybir.dt.float32
BF16 = mybir.dt.bfloat16
SILU = mybir.ActivationFunctionType.Silu


def build_nc(cfg):
    """Build the per-core Bass module. cfg: dict(BT=...)"""
    BT = cfg["BT"]

    NBT = BS // BT          # batch tiles per core
    NB_SUB = BT // P        # 128-row subtiles per batch tile
    KT = DM // P            # k-tiles for matmul 1
    NDI = DI // P           # d_inner chunks of 128
    NDM = DM // 512         # d_model chunks of 512
    H = BT // 512           # moving-dim chunks for matmul 1
    GRP = 4                 # psum banks used by M2 accumulation
    NGRP = NB_SUB // GRP
    DIG = 4                 # d_inner chunks per out_w DMA batch

    nc = bacc.Bacc("TRN2", target_bir_lowering=False, debug=False,
                   num_devices=N_CORES)

    xt_d = nc.dram_tensor("xt", [DM, BS], BF16, kind="ExternalInput").ap()
    iw_d = nc.dram_tensor("iw", [DM, DI], BF16, kind="ExternalInput").ap()
    ow_d = nc.dram_tensor("ow", [DI, DM], BF16, kind="ExternalInput").ap()
    c_d = nc.dram_tensor("cpb", [P, NDI], F32, kind="ExternalInput").ap()
    b_d = nc.dram_tensor("bpb", [P, NDI], F32, kind="ExternalInput").ap()
    ob_d = nc.dram_tensor("ob", [P, DM], F32, kind="ExternalInput").ap()
    out_d = nc.dram_tensor("out", [BS, DM], BF16, kind="ExternalOutput").ap()

    with tile.TileContext(nc) as tc:
        with (
            tc.tile_pool(name="const", bufs=1) as const,
            tc.tile_pool(name="xk", bufs=2) as xkp,
            tc.tile_pool(name="g", bufs=1) as gp,
            tc.tile_pool(name="iw", bufs=4) as iwp,
            tc.tile_pool(name="ow", bufs=6) as owp,
            tc.tile_pool(name="osb", bufs=2) as osbp,
            tc.tile_pool(name="psZ", bufs=4, space="PSUM") as psZ,
            tc.tile_pool(name="psO", bufs=4, space="PSUM") as psO,
        ):
            c_sb = const.tile([P, NDI], F32)
            nc.sync.dma_start(c_sb[:], c_d)
            b_sb = const.tile([P, NDI], F32)
            nc.sync.dma_start(b_sb[:], b_d)
            ob_sb = const.tile([P, DM], F32)
            nc.sync.dma_start(ob_sb[:], ob_d)

            g = gp.tile([P, NDI, BT], BF16)

            for t in range(NBT):
                # ---- xT tiles for this batch tile (per-kt for fine deps) --
                xk = []
                for kt in range(KT):
                    xt_t = xkp.tile([P, BT], BF16, tag=f"xk{kt}",
                                    name=f"xk{kt}")
                    # split the first tile's x stream across two queues so
                    # the cold-start di=0 pass isn't DMA-bound on one ring
                    eng = nc.gpsimd if (t > 0 or kt % 2 == 0) else nc.sync
                    eng.dma_start(
                        xt_t[:], xt_d[kt * P:(kt + 1) * P,
                                      t * BT:(t + 1) * BT])
                    xk.append(xt_t)

                # ---- phase M1: z^T = in_w^T @ x^T ; g = silu(z*c + b) ----
                for di in range(NDI):
                    iw_t = iwp.tile([P, KT, P], BF16)
                    nc.scalar.dma_start(
                        iw_t[:],
                        iw_d[:, di * P:(di + 1) * P].rearrange(
                            "(kt p) m -> p kt m", p=P))
                    zps = [psZ.tile([P, 512], F32, tag="zp", name=f"zp_{h}")
                           for h in range(H)]
                    for kt in range(KT):
                        for h in range(H):
                            nc.tensor.matmul(
                                zps[h][:],
                                iw_t[:, kt, :],
                                xk[kt][:, h * 512:(h + 1) * 512],
                                start=(kt == 0), stop=(kt == KT - 1))
                    for h in range(H):
                        nc.scalar.activation(
                            g[:, di, h * 512:(h + 1) * 512], zps[h][:], SILU,
                            bias=b_sb[:, di:di + 1], scale=c_sb[:, di:di + 1])

                # ---- phase M2: out = g^T @ out_w + out_b ----
                for dmc in range(NDM):
                    for grp in range(NGRP):
                        ops = [psO.tile([P, 512], F32, tag="ps_o",
                                        name=f"ops_{j}")
                               for j in range(GRP)]
                        NDG = NDI // DIG
                        osb = osbp.tile([P, GRP, 512], BF16)
                        r0 = t * BT + grp * GRP * P
                        for dg in range(NDG):
                            ow_t = owp.tile([P, DIG, 512], BF16)
                            nc.sync.dma_start(
                                ow_t[:],
                                ow_d[dg * DIG * P:(dg + 1) * DIG * P,
                                     dmc * 512:(dmc + 1) * 512].rearrange(
                                         "(s p) n -> p s n", p=P))
                            if dg < NDG - 1:
                                for s in range(DIG):
                                    di = dg * DIG + s
                                    for j in range(GRP):
                                        bs = grp * GRP + j
                                        nc.tensor.matmul(
                                            ops[j][:],
                                            g[:, di, bs * P:(bs + 1) * P],
                                            ow_t[:, s, :],
                                            start=(di == 0), stop=False)
                            else:
                                # last k-group j-major: bank j finishes all
                                # its matmuls early so its DVE drain + store
                                # overlap the remaining banks' matmuls and
                                # the bank is free for the next group sooner
                                for j in range(GRP):
                                    bs = grp * GRP + j
                                    for s in range(DIG):
                                        di = dg * DIG + s
                                        nc.tensor.matmul(
                                            ops[j][:],
                                            g[:, di, bs * P:(bs + 1) * P],
                                            ow_t[:, s, :],
                                            start=False,
                                            stop=(di == NDI - 1))
                                    nc.vector.tensor_tensor(
                                        osb[:, j, :], ops[j][:],
                                        ob_sb[:, dmc * 512:(dmc + 1) * 512],
                                        mybir.AluOpType.add)
                                    nc.scalar.dma_start(
                                        out_d[r0 + j * P:r0 + (j + 1) * P,
                                              dmc * 512:(dmc + 1) * 512],
                                        osb[:, j, :])
    nc.compile()
    return nc


_NC_CACHE = {}


def _get_nc(key):
    if key not in _NC_CACHE:
        cfg = dict(BT=key[0])
        _NC_CACHE[key] = build_nc(cfg)
    return _NC_CACHE[key]


CONFIG = (1024,)


def _softplus(v):
    return np.logaddexp(0.0, v)


def prepare_in_maps(inputs):
    """Host-side prep: per-core input dicts (shared by kernel() and tests)."""
    import ml_dtypes
    bf = ml_dtypes.bfloat16

    x = np.asarray(inputs["x"], dtype=np.float32)
    in_w = np.asarray(inputs["in_w"], dtype=np.float32)
    out_w = np.asarray(inputs["out_w"], dtype=np.float32)

    # host precompute of the per-channel SSM/conv collapse
    c = (np.asarray(inputs["conv_w"], np.float32)[:, -1]
         + _softplus(np.asarray(inputs["dt"], np.float32))
         * np.sum(np.asarray(inputs["B"], np.float32)
                  * np.asarray(inputs["C"], np.float32), -1)
         + np.asarray(inputs["Dp"], np.float32))
    b_eff = (np.asarray(inputs["in_b"], np.float32) * c
             + np.asarray(inputs["conv_b"], np.float32))

    # [128, DI//128] partition-major layouts for per-partition scale/bias
    c_pb = np.ascontiguousarray(c.reshape(DI // P, P).T)
    b_pb = np.ascontiguousarray(b_eff.reshape(DI // P, P).T)
    ob_rep = np.ascontiguousarray(
        np.broadcast_to(np.asarray(inputs["out_b"], np.float32), (P, DM)))

    iw_bf = in_w.astype(bf)
    ow_bf = out_w.astype(bf)
    x_bf = x.astype(bf)

    in_maps = []
    for i in range(N_CORES):
        in_maps.append({
            "xt": np.ascontiguousarray(x_bf[i * BS:(i + 1) * BS].T),
            "iw": iw_bf,
            "ow": ow_bf,
            "cpb": c_pb,
            "bpb": b_pb,
            "ob": ob_rep,
        })
    return in_maps


def kernel(x, in_w, in_b, conv_w, conv_b, A_log, B, C, Dp, dt, out_w, out_b):
    in_maps = prepare_in_maps(dict(
        x=x, in_w=in_w, in_b=in_b, conv_w=conv_w, conv_b=conv_b,
        A_log=A_log, B=B, C=C, Dp=Dp, dt=dt, out_w=out_w, out_b=out_b))
    nc = _get_nc(CONFIG)
    out = np.empty((B_FULL, DM), dtype=np.float32)
    try:
        res = run_bass_kernel_spmd(nc, in_maps, core_ids=list(range(N_CORES)))
        for i in range(N_CORES):
            out[i * BS:(i + 1) * BS] = res.results[i]["out"].astype(np.float32)
    except Exception:
        # The accelerator occasionally hits a transient unrecoverable fault
        # that poisons this process's PJRT client; a fresh process recovers.
        # Retry the device execution in a subprocess.
        _run_in_subprocess(in_maps, out)
    return out


def _run_in_subprocess(in_maps, out):
    import pickle
    import subprocess
    import sys
    import tempfile

    with tempfile.TemporaryDirectory() as td:
        in_path = f"{td}/in.pkl"
        out_path = f"{td}/out.npy"
        with open(in_path, "wb") as f:
            pickle.dump({"config": CONFIG, "in_maps": in_maps}, f,
                        protocol=pickle.HIGHEST_PROTOCOL)
        for attempt in range(3):
            r = subprocess.run(
                [sys.executable, __file__, "--worker", in_path, out_path],
                capture_output=True)
            if r.returncode == 0:
                break
            if attempt == 2:
                raise RuntimeError(
                    f"device worker failed 3x: {r.stderr[-2000:]!r}")
        out[:] = np.load(out_path)


def _worker_main(in_path, out_path):
    import pickle
    with open(in_path, "rb") as f:
        job = pickle.load(f)
    nc = _get_nc(tuple(job["config"]))
    res = run_bass_kernel_spmd(nc, job["in_maps"],
                               core_ids=list(range(N_CORES)))
    out = np.empty((B_FULL, DM), dtype=np.float32)
    for i in range(N_CORES):
        out[i * BS:(i + 1) * BS] = res.results[i]["out"].astype(np.float32)
    np.save(out_path, out)


if __name__ == "__main__":
    import sys as _sys
    if len(_sys.argv) == 4 and _sys.argv[1] == "--worker":
        _worker_main(_sys.argv[2], _sys.argv[3])


# revision 8
# speedup vs baseline: 1.1021x; 1.0012x over previous
"""Trainium2 Bass kernel for a dense (length-1 sequence) Mamba block.

The reference computation reduces algebraically to:
    z   = x @ in_w                                  # (B, d_inner)
    g   = silu(z * c + b_eff)                       # per-channel scale/bias
    out = g @ out_w + out_b                         # (B, d_model)
with
    c     = conv_w[:, -1] + softplus(dt) * sum(B*C, -1) + Dp
    b_eff = (in_b * c) + conv_b
(c, b_eff are tiny per-channel vectors, computed on host.)

Strategy: data-parallel over 8 NeuronCores (batch 32768 -> 8 x 4096).
All matmul operands are bf16 (validated: end-to-end rel err ~4e-3 vs the
2e-2 budget), which enables fast-weight-load on every LDWEIGHTS and
halves HBM traffic. x is transposed on the HOST so the kernel streams
xT [d_model, batch] tiles straight from DRAM -- no PE transposes at all.

Per core, batch is processed in tiles of BT rows:
  M1: z^T[di, b] accumulated over d_model with in_w tiles stationary;
      Silu fused on ScalarE with per-partition scale/bias -> g [di, b]
  M2: out[b, dm] accumulated over d_inner with g slices stationary and
      out_w tiles moving (natural output layout); out_b added on the
      PSUM drain; result stored/DMA'd as bf16 and upcast on host.
"""

import numpy as np

import concourse.bass as bass
import concourse.tile as tile
from concourse import bacc, mybir
from concourse.bass_utils import run_bass_kernel_spmd

P = 128
B_FULL = 32768
DM = 2048
DI = 4096
N_CORES = 8
BS = B_FULL // N_CORES  # rows per core

F32 = mybir.dt.float32
BF16 = mybir.dt.bfloat16
SILU = mybir.ActivationFunctionType.Silu


def build_nc(cfg):
    """Build the per-core Bass module. cfg: dict(BT=...)"""
    BT = cfg["BT"]

    NBT = BS // BT          # batch tiles per core
    NB_SUB = BT // P        # 128-row subtiles per batch tile
    KT = DM // P            # k-tiles for matmul 1
    NDI = DI // P           # d_inner chunks of 128
    NDM = DM // 512         # d_model chunks of 512
    H = BT // 512           # moving-dim chunks for matmul 1
    GRP = 4                 # psum banks used by M2 accumulation
    NGRP = NB_SUB // GRP
    DIG = 4                 # d_inner chunks per out_w DMA batch

    nc = bacc.Bacc("TRN2", target_bir_lowering=False, debug=False,
                   num_devices=N_CORES)

    xt_d = nc.dram_tensor("xt", [DM, BS], BF16, kind="ExternalInput").ap()
    iw_d = nc.dram_tensor("iw", [DM, DI], BF16, kind="ExternalInput").ap()
    ow_d = nc.dram_tensor("ow", [DI, DM], BF16, kind="ExternalInput").ap()
    c_d = nc.dram_tensor("cpb", [P, NDI], F32, kind="ExternalInput").ap()
    b_d = nc.dram_tensor("bpb", [P, NDI], F32, kind="ExternalInput").ap()
    ob_d = nc.dram_tensor("ob", [P, DM], F32, kind="ExternalInput").ap()
    out_d = nc.dram_tensor("out", [BS, DM], BF16, kind="ExternalOutput").ap()

    with tile.TileContext(nc) as tc:
        with (
            tc.tile_pool(name="const", bufs=1) as const,
            tc.tile_pool(name="xk", bufs=2) as xkp,
            tc.tile_pool(name="g", bufs=1) as gp,
            tc.tile_pool(name="iw", bufs=4) as iwp,
            tc.tile_pool(name="ow", bufs=6) as owp,
            tc.tile_pool(name="osb", bufs=2) as osbp,
            tc.tile_pool(name="psZ", bufs=4, space="PSUM") as psZ,
            tc.tile_pool(name="psO", bufs=4, space="PSUM") as psO,
        ):
            # scale/bias consts on the scalar ring (tiny, needed by the first
            # ACT at ~7us); out_b is only needed at the first M2 drain, so
            # its 1MB load is emitted after the startup-critical t=0 DMAs
            c_sb = const.tile([P, NDI], F32)
            nc.scalar.dma_start(c_sb[:], c_d)
            b_sb = const.tile([P, NDI], F32)
            nc.scalar.dma_start(b_sb[:], b_d)
            ob_sb = const.tile([P, DM], F32)

            g = gp.tile([P, NDI, BT], BF16)

            # t=0 x stream engine per kt: spread 7/7/2 across the gpsimd /
            # sync / scalar rings so the cold-start di=0 pass isn't bound on
            # one ring; scalar gets the last-consumed kts (it also carries iw)
            t0_eng = {}
            for kt in range(KT):
                t0_eng[kt] = nc.scalar if kt >= KT - 2 else (
                    nc.gpsimd if kt % 2 == 0 else nc.sync)

            iw_first = None
            for t in range(NBT):
                # ---- phase M1: z^T = in_w^T @ x^T ; g = silu(z*c + b) ----
                if t == 0:
                    # hoist the first in_w tile DMA ahead of the x stream on
                    # the scalar ring: di=0 needs it immediately
                    iw_first = iwp.tile([P, KT, P], BF16)
                    nc.scalar.dma_start(
                        iw_first[:],
                        iw_d[:, 0:P].rearrange("(kt p) m -> p kt m", p=P))

                # ---- xT tiles for this batch tile (per-kt for fine deps) --
                xk = []
                for kt in range(KT):
                    xt_t = xkp.tile([P, BT], BF16, tag=f"xk{kt}",
                                    name=f"xk{kt}")
                    eng = t0_eng[kt] if t == 0 else nc.gpsimd
                    eng.dma_start(
                        xt_t[:], xt_d[kt * P:(kt + 1) * P,
                                      t * BT:(t + 1) * BT])
                    xk.append(xt_t)

                if t == 0:
                    nc.gpsimd.dma_start(ob_sb[:], ob_d)

                for di in range(NDI):
                    if t == 0 and di == 0:
                        iw_t = iw_first
                    else:
                        iw_t = iwp.tile([P, KT, P], BF16)
                        nc.scalar.dma_start(
                            iw_t[:],
                            iw_d[:, di * P:(di + 1) * P].rearrange(
                                "(kt p) m -> p kt m", p=P))
                    zps = [psZ.tile([P, 512], F32, tag="zp", name=f"zp_{h}")
                           for h in range(H)]
                    for kt in range(KT):
                        for h in range(H):
                            nc.tensor.matmul(
                                zps[h][:],
                                iw_t[:, kt, :],
                                xk[kt][:, h * 512:(h + 1) * 512],
                                start=(kt == 0), stop=(kt == KT - 1))
                    for h in range(H):
                        nc.scalar.activation(
                            g[:, di, h * 512:(h + 1) * 512], zps[h][:], SILU,
                            bias=b_sb[:, di:di + 1], scale=c_sb[:, di:di + 1])

                # ---- phase M2: out = g^T @ out_w + out_b ----
                for dmc in range(NDM):
                    for grp in range(NGRP):
                        ops = [psO.tile([P, 512], F32, tag="ps_o",
                                        name=f"ops_{j}")
                               for j in range(GRP)]
                        NDG = NDI // DIG
                        osb = osbp.tile([P, GRP, 512], BF16)
                        r0 = t * BT + grp * GRP * P
                        for dg in range(NDG):
                            ow_t = owp.tile([P, DIG, 512], BF16)
                            nc.sync.dma_start(
                                ow_t[:],
                                ow_d[dg * DIG * P:(dg + 1) * DIG * P,
                                     dmc * 512:(dmc + 1) * 512].rearrange(
                                         "(s p) n -> p s n", p=P))
                            if dg < NDG - 1:
                                for s in range(DIG):
                                    di = dg * DIG + s
                                    for j in range(GRP):
                                        bs = grp * GRP + j
                                        nc.tensor.matmul(
                                            ops[j][:],
                                            g[:, di, bs * P:(bs + 1) * P],
                                            ow_t[:, s, :],
                                            start=(di == 0), stop=False)
                            else:
                                # last k-group j-major: bank j finishes all
                                # its matmuls early so its DVE drain + store
                                # overlap the remaining banks' matmuls and
                                # the bank is free for the next group sooner
                                for j in range(GRP):
                                    bs = grp * GRP + j
                                    for s in range(DIG):
                                        di = dg * DIG + s
                                        nc.tensor.matmul(
                                            ops[j][:],
                                            g[:, di, bs * P:(bs + 1) * P],
                                            ow_t[:, s, :],
                                            start=False,
                                            stop=(di == NDI - 1))
                                    nc.vector.tensor_tensor(
                                        osb[:, j, :], ops[j][:],
                                        ob_sb[:, dmc * 512:(dmc + 1) * 512],
                                        mybir.AluOpType.add)
                                    nc.scalar.dma_start(
                                        out_d[r0 + j * P:r0 + (j + 1) * P,
                                              dmc * 512:(dmc + 1) * 512],
                                        osb[:, j, :])
    nc.compile()
    return nc


_NC_CACHE = {}


def _get_nc(key):
    if key not in _NC_CACHE:
        cfg = dict(BT=key[0])
        _NC_CACHE[key] = build_nc(cfg)
    return _NC_CACHE[key]


CONFIG = (1024,)


def _softplus(v):
    return np.logaddexp(0.0, v)


def prepare_in_maps(inputs):
    """Host-side prep: per-core input dicts (shared by kernel() and tests)."""
    import ml_dtypes
    bf = ml_dtypes.bfloat16

    x = np.asarray(inputs["x"], dtype=np.float32)
    in_w = np.asarray(inputs["in_w"], dtype=np.float32)
    out_w = np.asarray(inputs["out_w"], dtype=np.float32)

    # host precompute of the per-channel SSM/conv collapse
    c = (np.asarray(inputs["conv_w"], np.float32)[:, -1]
         + _softplus(np.asarray(inputs["dt"], np.float32))
         * np.sum(np.asarray(inputs["B"], np.float32)
                  * np.asarray(inputs["C"], np.float32), -1)
         + np.asarray(inputs["Dp"], np.float32))
    b_eff = (np.asarray(inputs["in_b"], np.float32) * c
             + np.asarray(inputs["conv_b"], np.float32))

    # [128, DI//128] partition-major layouts for per-partition scale/bias
    c_pb = np.ascontiguousarray(c.reshape(DI // P, P).T)
    b_pb = np.ascontiguousarray(b_eff.reshape(DI // P, P).T)
    ob_rep = np.ascontiguousarray(
        np.broadcast_to(np.asarray(inputs["out_b"], np.float32), (P, DM)))

    iw_bf = in_w.astype(bf)
    ow_bf = out_w.astype(bf)
    x_bf = x.astype(bf)

    in_maps = []
    for i in range(N_CORES):
        in_maps.append({
            "xt": np.ascontiguousarray(x_bf[i * BS:(i + 1) * BS].T),
            "iw": iw_bf,
            "ow": ow_bf,
            "cpb": c_pb,
            "bpb": b_pb,
            "ob": ob_rep,
        })
    return in_maps


def kernel(x, in_w, in_b, conv_w, conv_b, A_log, B, C, Dp, dt, out_w, out_b):
    in_maps = prepare_in_maps(dict(
        x=x, in_w=in_w, in_b=in_b, conv_w=conv_w, conv_b=conv_b,
        A_log=A_log, B=B, C=C, Dp=Dp, dt=dt, out_w=out_w, out_b=out_b))
    nc = _get_nc(CONFIG)
    out = np.empty((B_FULL, DM), dtype=np.float32)
    try:
        res = run_bass_kernel_spmd(nc, in_maps, core_ids=list(range(N_CORES)))
        for i in range(N_CORES):
            out[i * BS:(i + 1) * BS] = res.results[i]["out"].astype(np.float32)
    except Exception:
        # The accelerator occasionally hits a transient unrecoverable fault
        # that poisons this process's PJRT client; a fresh process recovers.
        # Retry the device execution in a subprocess.
        _run_in_subprocess(in_maps, out)
    return out


def _run_in_subprocess(in_maps, out):
    import pickle
    import subprocess
    import sys
    import tempfile

    with tempfile.TemporaryDirectory() as td:
        in_path = f"{td}/in.pkl"
        out_path = f"{td}/out.npy"
        with open(in_path, "wb") as f:
            pickle.dump({"config": CONFIG, "in_maps": in_maps}, f,
                        protocol=pickle.HIGHEST_PROTOCOL)
        for attempt in range(3):
            r = subprocess.run(
                [sys.executable, __file__, "--worker", in_path, out_path],
                capture_output=True)
            if r.returncode == 0:
                break
            if attempt == 2:
                raise RuntimeError(
                    f"device worker failed 3x: {r.stderr[-2000:]!r}")
        out[:] = np.load(out_path)


def _worker_main(in_path, out_path):
    import pickle
    with open(in_path, "rb") as f:
        job = pickle.load(f)
    nc = _get_nc(tuple(job["config"]))
    res = run_bass_kernel_spmd(nc, job["in_maps"],
                               core_ids=list(range(N_CORES)))
    out = np.empty((B_FULL, DM), dtype=np.float32)
    for i in range(N_CORES):
        out[i * BS:(i + 1) * BS] = res.results[i]["out"].astype(np.float32)
    np.save(out_path, out)


if __name__ == "__main__":
    import sys as _sys
    if len(_sys.argv) == 4 and _sys.argv[1] == "--worker":
        _worker_main(_sys.argv[2], _sys.argv[3])
